# revision 2
# baseline (speedup 1.0000x reference)
"""Self-contained Trainium2 Bass kernel for the 3-layer GAT + graph readout
(nn_GAT_36361193128013). 8-core SPMD over one trn2 chip:

- graph-aligned node sharding (64 graphs / ~6250 nodes per core), so the
  segment readout never crosses cores;
- per-layer dense phase (x @ W, attention coefficients) into a 256B/row bf16
  node table [h bf16(64) | a_s f32(4) | a_d f32(4)], AllGather-replicated
  across the 8 cores (halo exchange);
- edge phase over a uniform window/tile structure (50 windows x 36 tiles of
  128 dst-sorted edges per core): per-tile indirect row gathers of h|a_s by
  src, a_d expansion via select-reduce against the one-hot tile matrix,
  softmax (exp without max-subtraction -- mathematically identical, f32-safe)
  on ACT/DVE, and the segment scatter-add as one-hot matmuls accumulating
  [128 nodes, 64ch + 4 denom] in PSUM on TensorE;
- graph readout via resettable segmented scans (sum/max) + indirect
  extraction at graph boundaries + the final [48->1] projection on DVE.

kernel(**inputs) takes the FULL inputs (x, edge_index, batch_index, weights)
and returns the FULL [512, 1] float32 output.
"""
import numpy as np
import ml_dtypes
import concourse.bass as bass
import concourse.mybir as mybir
import concourse.tile as tile
from concourse.bass_utils import run_bass_kernel_spmd

dt = mybir.dt
AF = mybir.ActivationFunctionType
ALU = mybir.AluOpType
AX = mybir.AxisListType


H, C = 4, 16
HC = H * C
N = 50000
G = 512
NC = 8
GPC = G // NC          # graphs per core
W = 128                # dst nodes per window
NLOC = 6400            # padded local nodes per core (multiple of 128)
NWIN = NLOC // W       # 50
EA = 35                # tiles per window (merged list; int32 rows need no halves)
EB = 0
T = EA + EB            # 35 tiles per window
NTAB = NC * NLOC       # 51200 table rows
HALF = NTAB // 2       # 25600
ROWE = 128             # bf16 elems per table row (256B)
PAD_DSTREL = 200.0

# edata int16 layout per window: [srcrow int32 | dstabs int32 | dstrel f32] (72 each)
SR_COLS = T * 2            # 36 int32 = 72 i16
DA_COLS = T * 2
DR_COLS = T * 2            # 36 f32 = 72 i16
EDATA_COLS = SR_COLS + DA_COLS + DR_COLS  # 216


def wrap16(idx_list, ncols):
    """Pack index list into dma_gather layout [128, ncols] int16:
    index k at [k%16, k//16], replicated to all 8 groups of 16 partitions."""
    arr = np.zeros((16, ncols), np.int16)
    k = np.arange(len(idx_list))
    arr[k % 16, k // 16] = idx_list
    return np.tile(arr, (8, 1))


def prep(x, edge_index, batch_index):
    src = edge_index[0].astype(np.int64)
    dst = edge_index[1].astype(np.int64)
    bi = batch_index.astype(np.int64)

    gstart = np.searchsorted(bi, np.arange(0, G + 1, GPC))  # node start per core
    core_of = np.searchsorted(gstart, np.arange(N), side="right") - 1
    loc_of = np.arange(N) - gstart[core_of]
    row_of = (core_of * NLOC + loc_of).astype(np.int64)

    per_core = []
    for c in range(NC):
        ns, ne = gstart[c], gstart[c + 1]
        nloc = ne - ns
        m = (dst >= ns) & (dst < ne)
        e_dst = dst[m] - ns
        e_row = row_of[src[m]]
        # self loops
        e_dst = np.concatenate([e_dst, np.arange(nloc)])
        e_row = np.concatenate([e_row, row_of[np.arange(ns, ne)]])
        order = np.argsort(e_dst, kind="stable")
        e_dst = e_dst[order]
        e_row = e_row[order]

        lo_m = np.ones(len(e_row), bool)  # merged: no half split
        win = e_dst // W

        # per window, build slot arrays: EA*128 lo slots then EB*128 hi slots
        gidx_lo = np.zeros((NWIN, EA * 128), np.int64)     # pad idx 0
        gidx_hi = np.zeros((NWIN, EB * 128), np.int64)
        dstrel = np.full((NWIN, T * 128), PAD_DSTREL, np.float32)
        for w in range(NWIN):
            wm = win == w
            for half, (gi, off, cap) in enumerate(
                ((gidx_lo, 0, EA * 128), (gidx_hi, EA * 128, EB * 128))
            ):
                hm = wm & (lo_m if half == 0 else ~lo_m)
                rows = e_row[hm] - (0 if half == 0 else HALF)
                drs = e_dst[hm] - w * W
                n = len(rows)
                assert n <= cap, (c, w, half, n, cap)
                gi[w, :n] = rows
                dstrel[w, off : off + n] = drs
        per_core.append(
            dict(nloc=nloc, ns=ns, gidx_lo=gidx_lo, gidx_hi=gidx_hi, dstrel=dstrel)
        )

    # assemble per-core device inputs
    inputs = []
    for c in range(NC):
        pc = per_core[c]
        nloc, ns = pc["nloc"], pc["ns"]
        # xT [128, NLOC]
        xT = np.zeros((128, NLOC), np.float32)
        xT[:, :nloc] = x[ns : ns + nloc].T
        # edata [128, NWIN*EDATA_COLS] int16
        ed = np.zeros((NWIN, 128, EDATA_COLS), np.int16)
        for w in range(NWIN):
            # src rows: tiles 0..EA-1 from lo list, EA..T-1 from hi list (+HALF)
            srl = pc["gidx_lo"][w].reshape(EA, 128)
            srh = pc["gidx_hi"][w].reshape(EB, 128) + HALF
            sr = np.concatenate([srl, srh], 0).T.astype(np.int32)  # [128, T]
            ed[w, :, :SR_COLS] = np.ascontiguousarray(sr).view(np.int16).reshape(128, SR_COLS)
            # dstrel in edge-partition layout [p, t] (edge slot k = t*128+p)
            dr = np.ascontiguousarray(
                pc["dstrel"][w].reshape(T, 128).T
            ).astype(np.float32)  # [128, T]
            da = (np.clip(dr, 0, W - 1).astype(np.int32) + w * W)  # abs local node id
            ed[w, :, SR_COLS : SR_COLS + DA_COLS] = (
                np.ascontiguousarray(da).view(np.int16).reshape(128, DA_COLS)
            )
            ed[w, :, SR_COLS + DA_COLS :] = dr.view(np.int16).reshape(128, DR_COLS)
        edata = ed.transpose(1, 0, 2).reshape(128, NWIN * EDATA_COLS).copy()

        # readout: graph boundaries within the core
        gs = gstart[c] + 0
        bounds = np.searchsorted(bi, np.arange(c * GPC, (c + 1) * GPC + 1)) - gs
        # scan vectors [16, NLOC]
        z = np.ones(NLOC, np.float32)
        r = np.zeros(NLOC, np.float32)
        z[bounds[:-1]] = 0.0
        r[bounds[:-1]] = -1e30
        z16 = np.tile(z, (16, 1))
        r16 = np.tile(r, (16, 1))
        # gend: index of last node of each graph
        gends = (bounds[1:] - 1).astype(np.int32).reshape(GPC, 1)
        cnt = np.diff(bounds).astype(np.float32)
        invcnt64 = (1.0 / np.maximum(cnt, 1.0)).astype(np.float32).reshape(GPC, 1)

        inputs.append(
            dict(
                xT=xT,
                edata=edata,
                z16=z16,
                r16=r16,
                gends=gends,
                invcnt64=invcnt64,
            )
        )
    return inputs, gstart


def prep_params(d):
    """Replicated parameter tensors (same for all cores)."""
    out = {}
    iota = np.tile(np.arange(W, dtype=np.float32), (128, 1))
    out["iota"] = iota  # [128, 128]
    for l, fin in ((1, 128), (2, HC), (3, HC)):
        Wl = d[f"W{l}"].astype(np.float32)           # [fin, 64]
        out[f"W{l}"] = Wl
        out[f"asrep{l}"] = np.tile(d[f"as{l}"].reshape(1, HC), (128, 1)).astype(np.float32)
        out[f"adrep{l}"] = np.tile(d[f"ad{l}"].reshape(1, HC), (128, 1)).astype(np.float32)
        out[f"brep{l}"] = np.tile(d[f"b{l}"].reshape(1, HC), (128, 1)).astype(np.float32)
    out["Wout"] = d["Wout"].astype(np.float32)       # [48, 1]
    out["bout"] = np.float32(d["bout"][0])
    return out



def null_input_decls():
    """Inputs the timing-floor null kernel should also upload (largest bufs)."""
    return [
        ("xT1", [128, NLOC], dt.float32),
        ("edata", [128, NWIN * EDATA_COLS], dt.int16),
    ]


_ctr = [0]


def split_waits(nc):
    for _name, bbwrap in nc.bb_map.items():
        bb = bbwrap.bb if hasattr(bbwrap, "bb") else bbwrap
        insts = bb.instructions
        i = 0
        while i < len(insts):
            inst = insts[i]
            si = inst.sync_info
            if si is not None and si.on_wait and len(si.on_wait) > 1:
                waits = list(si.on_wait)
                si.on_wait = waits[:1]
                rest = waits[1:]
                for w in rest:
                    _ctr[0] += 1
                    nop = mybir.InstNoOp(name=f"splitw-{_ctr[0]}", ins=[], outs=[])
                    nop.engine = inst.engine
                    nop.sync_info = mybir.SyncInfo(on_wait=[w], on_update=[])
                    nc.register_instruction(nop)
                    insts.insert(i, nop)
                    i += 1
            i += 1


def apply():
    pass



dt = mybir.dt
AF = mybir.ActivationFunctionType
ALU = mybir.AluOpType
AX = mybir.AxisListType

ECOL = EDATA_COLS


def build(n_cores=8):
    nc = bass.Bass(target_bir_lowering=False)

    # inputs
    xT1 = nc.declare_dram_parameter("xT1", [128, NLOC], dt.float32, isOutput=False)
    edata = nc.declare_dram_parameter("edata", [128, NWIN * ECOL], dt.int16, isOutput=False)
    z16d = nc.declare_dram_parameter("z16", [16, NLOC], dt.float32, isOutput=False)
    r16d = nc.declare_dram_parameter("r16", [16, NLOC], dt.float32, isOutput=False)
    gendd = nc.declare_dram_parameter("gends", [64, 1], dt.int32, isOutput=False)
    invcd = nc.declare_dram_parameter("invcnt64", [64, 1], dt.float32, isOutput=False)
    Wd = {
        1: nc.declare_dram_parameter("W1", [128, 64], dt.float32, isOutput=False),
        2: nc.declare_dram_parameter("W2", [64, 64], dt.float32, isOutput=False),
        3: nc.declare_dram_parameter("W3", [64, 64], dt.float32, isOutput=False),
    }
    asd, add_, bd = {}, {}, {}
    for l in (1, 2, 3):
        asd[l] = nc.declare_dram_parameter(f"asrep{l}", [128, 64], dt.float32, isOutput=False)
        add_[l] = nc.declare_dram_parameter(f"adrep{l}", [128, 64], dt.float32, isOutput=False)
        bd[l] = nc.declare_dram_parameter(f"brep{l}", [128, 64], dt.float32, isOutput=False)
    iotad = nc.declare_dram_parameter("iota", [128, 128], dt.float32, isOutput=False)
    idf32d = nc.declare_dram_parameter("idf32", [128, 128], dt.float32, isOutput=False)
    idbfd = nc.declare_dram_parameter("idbf", [128, 128], dt.bfloat16, isOutput=False)
    wmaxd = nc.declare_dram_parameter("wmaxr", [64, 16], dt.float32, isOutput=False)
    wmeand = nc.declare_dram_parameter("wmeanr", [64, 16], dt.float32, isOutput=False)
    wsumd = nc.declare_dram_parameter("wsumr", [64, 16], dt.float32, isOutput=False)
    boutd = nc.declare_dram_parameter("boutr", [64, 1], dt.float32, isOutput=False)
    outd = nc.declare_dram_parameter("out", [64, 1], dt.float32, isOutput=True)

    with tile.TileContext(nc) as tc:
      with tc.tile_pool(name="outer", bufs=1) as op_:
        hmT = op_.tile([16, NLOC], dt.float32, tag="hmT")
        with (
            tc.tile_pool(name="const", bufs=1) as cp,
            tc.tile_pool(name="pin", bufs=1) as pin,
            tc.tile_pool(name="work", bufs=2) as wp,
            tc.tile_pool(name="sml", bufs=3) as sp,
            tc.tile_pool(name="ps_big", bufs=2, space="PSUM") as pb,
            tc.tile_pool(name="ps_acc", bufs=2, space="PSUM") as pa,
            tc.tile_pool(name="ps_ade", bufs=1, space="PSUM") as pd,
            tc.tile_pool(name="ps_misc", bufs=2, space="PSUM") as pm_,
            tc.tile_pool(name="dram", bufs=1, space="DRAM") as dp,
        ):
            # consts to SBUF
            def ld(dram, shape, dtp):
                t = cp.tile(shape, dtp, tag=dram.name)
                nc.sync.dma_start(t[:], dram[:])
                return t

            Ws = {l: ld(Wd[l], Wd[l].shape, dt.float32) for l in (1, 2, 3)}
            ass = {l: ld(asd[l], [128, 64], dt.float32) for l in (1, 2, 3)}
            ads = {l: ld(add_[l], [128, 64], dt.float32) for l in (1, 2, 3)}
            bs = {l: ld(bd[l], [128, 64], dt.float32) for l in (1, 2, 3)}
            iota = ld(iotad, [128, 128], dt.float32)
            idf32 = ld(idf32d, [128, 128], dt.float32)
            idbf = ld(idbfd, [128, 128], dt.bfloat16)

            # pinned state
            xT2 = pin.tile([64, NLOC], dt.float32, tag="xT2")

            # DRAM internal
            myrows = dp.tile([NLOC, 128], dt.bfloat16, tag="myrows")
            table = dp.tile([NTAB, 128], dt.bfloat16, tag="table")
            adtab = dp.tile([4, NLOC], dt.bfloat16, tag="adtab")

            def dense_phase(l):
                """h = x @ W_l per 128-node chunk -> myrows + adq; then AllGather."""
                K = 128 if l == 1 else 64
                for w in range(NWIN):
                    if l == 1:
                        xc = wp.tile([128, 128], dt.float32, tag="xc")
                        nc.sync.dma_start(xc[:], xT1[:, w * 128 : (w + 1) * 128])
                        lhsT = xc[:]
                    else:
                        lhsT = xT2[:, w * 128 : (w + 1) * 128]
                    h_ps = pm_.tile([128, 64], dt.float32, tag="mps", space="PSUM")
                    nc.tensor.matmul(out=h_ps[:], lhsT=lhsT, rhs=Ws[l][:], start=True, stop=True)
                    h_sb = sp.tile([128, 64], dt.float32, tag="h_sb")
                    nc.vector.tensor_copy(h_sb[:], h_ps[:])
                    # a_s / a_d
                    tmp = sp.tile([128, 64], dt.float32, tag="astmp")
                    asad = sp.tile([128, 8], dt.float32, tag="asad")
                    nc.vector.tensor_mul(tmp[:], h_sb[:], ass[l][:])
                    nc.vector.tensor_reduce(
                        asad[:, 0:4], tmp[:].rearrange("p (h c) -> p h c", c=16),
                        axis=AX.X, op=ALU.add,
                    )
                    nc.vector.tensor_mul(tmp[:], h_sb[:], ads[l][:])
                    nc.vector.tensor_reduce(
                        asad[:, 4:8], tmp[:].rearrange("p (h c) -> p h c", c=16),
                        axis=AX.X, op=ALU.add,
                    )
                    rowt = sp.tile([128, 128], dt.bfloat16, tag="rowt")
                    nc.vector.tensor_copy(rowt[:, 0:64], h_sb[:])
                    nc.vector.tensor_copy(rowt[:, 64:80].bitcast(dt.float32), asad[:])
                    ad_bf = sp.tile([128, 4], dt.bfloat16, tag="ad_bf")
                    nc.vector.tensor_copy(ad_bf[:], asad[:, 4:8])
                    nc.sync.dma_start(
                        adtab[:, w * 128 : (w + 1) * 128].transpose([1, 0]), ad_bf[:]
                    )
                    nc.sync.dma_start(myrows[w * 128 : (w + 1) * 128, :], rowt[:])
                nc.gpsimd.collective_compute(
                    "AllGather",
                    ALU.bypass,
                    replica_groups=[list(range(n_cores))],
                    ins=[myrows[:].opt()],
                    outs=[table[:].opt()],
                )

            def edge_phase(l):
                for w in range(NWIN):
                    ew = wp.tile([128, ECOL], dt.int16, tag="ew")
                    nc.sync.dma_start(ew[:], edata[:, w * ECOL : (w + 1) * ECOL])
                    srcrow = ew[:, 0 : SR_COLS].bitcast(dt.int32)
                    hsrc = wp.tile([128, T, 128], dt.bfloat16, tag="hsrc")
                    for t in range(T):
                        nc.gpsimd.indirect_dma_start(
                            out=hsrc[:, t, :], out_offset=None, in_=table[:],
                            in_offset=bass.IndirectOffsetOnAxis(
                                ap=srcrow[:, t : t + 1], axis=0
                            ),
                        )
                    drel = ew[:, SR_COLS + DA_COLS : ECOL].bitcast(dt.float32)
                    S3 = wp.tile([128, T, W], dt.bfloat16, tag="S3")
                    nc.vector.tensor_tensor(
                        out=S3[:],
                        in0=drel.to_broadcast([128, T, W]),
                        in1=iota[:].unsqueeze(1).to_broadcast([128, T, W]),
                        op=ALU.is_equal,
                    )
                    adrep = wp.tile([128, 4, 128], dt.bfloat16, tag="adrep")
                    nc.sync.dma_start(
                        adrep[:],
                        adtab[:, w * 128 : (w + 1) * 128]
                        .unsqueeze(0)
                        .to_broadcast([128, 4, 128]),
                    )
                    ade = wp.tile([128, T, 4], dt.float32, tag="ade")
                    tmph = wp.tile([128, T, 2, 128], dt.bfloat16, tag="tmph")
                    for h in range(0, 4, 2):
                        nc.vector.tensor_tensor(
                            out=tmph[:],
                            in0=S3[:].unsqueeze(2).to_broadcast([128, T, 2, 128]),
                            in1=adrep[:, h : h + 2, :].unsqueeze(1).to_broadcast(
                                [128, T, 2, 128]
                            ),
                            op=ALU.mult,
                        )
                        nc.vector.tensor_reduce(
                            ade[:, :, h : h + 2], tmph[:], axis=AX.X, op=ALU.add,
                        )
                    e_sb = sp.tile([128, T * 4], dt.float32, tag="e_sb")
                    nc.vector.tensor_tensor(
                        out=e_sb[:].rearrange("p (t f) -> p t f", f=4),
                        in0=hsrc[:, :, 64:72].bitcast(dt.float32),
                        in1=ade[:],
                        op=ALU.add,
                    )
                    nc.vector.scalar_tensor_tensor(
                        out=e_sb[:], in0=e_sb[:], scalar=0.2, in1=e_sb[:],
                        op0=ALU.mult, op1=ALU.max,
                    )
                    wmsg = wp.tile([128, T, 72], dt.bfloat16, tag="wmsg")
                    nc.scalar.activation(
                        wmsg[:, :, 64:68], e_sb[:].rearrange("p (t f) -> p t f", f=4),
                        AF.Exp,
                    )
                    nc.vector.tensor_tensor(
                        out=wmsg[:, :, 0:64].rearrange("p t (h c) -> p t h c", c=16),
                        in0=hsrc[:, :, 0:64].rearrange("p t (h c) -> p t h c", c=16),
                        in1=wmsg[:, :, 64:68].unsqueeze(3).to_broadcast([128, T, 4, 16]),
                        op=ALU.mult,
                    )
                    out_ps = pa.tile([128, 68], dt.float32, tag="out_ps", space="PSUM")
                    for t in range(T):
                        nc.tensor.matmul(
                            out=out_ps[:],
                            lhsT=S3[:, t, :],
                            rhs=wmsg[:, t, 0:68],
                            start=(t == 0), stop=(t == T - 1),
                        )
                    # node phase
                    sg = sp.tile([128, 4], dt.float32, tag="sg")
                    nc.vector.tensor_scalar_add(sg[:], out_ps[:, 64:68], 1e-30)
                    rs = sp.tile([128, 4], dt.float32, tag="rs")
                    nc.vector.reciprocal(rs[:], sg[:])
                    xn = sp.tile([128, 64], dt.float32, tag="xn")
                    nc.vector.tensor_tensor(
                        out=xn[:].rearrange("p (h c) -> p h c", c=16),
                        in0=out_ps[:, 0:64].rearrange("p (h c) -> p h c", c=16),
                        in1=rs[:].unsqueeze(2).to_broadcast([128, 4, 16]),
                        op=ALU.mult,
                    )
                    nc.vector.tensor_add(xn[:], xn[:], bs[l][:])
                    nc.scalar.activation(xn[:], xn[:], AF.Tanh)
                    if l < 3:
                        xt_ps = pm_.tile([64, 128], dt.float32, tag="mps", space="PSUM")
                        nc.tensor.transpose(out=xt_ps[:], in_=xn[:], identity=idf32[:])
                        nc.vector.tensor_copy(xT2[:, w * 128 : (w + 1) * 128], xt_ps[:])
                    else:
                        hm = sp.tile([128, 16], dt.float32, tag="hm")
                        nc.vector.tensor_reduce(
                            hm[:], xn[:].rearrange("p (h c) -> p c h", c=16),
                            axis=AX.X, op=ALU.add,
                        )
                        hm_ps = pm_.tile([16, 128], dt.float32, tag="mps", space="PSUM")
                        nc.tensor.transpose(out=hm_ps[:], in_=hm[:], identity=idf32[:])
                        nc.vector.tensor_copy(hmT[:, w * 128 : (w + 1) * 128], hm_ps[:])

            dense_phase(1)
            edge_phase(1)
            dense_phase(2)
            edge_phase(2)
            dense_phase(3)
            edge_phase(3)

        with (
            tc.tile_pool(name="ro", bufs=1) as cp,
            tc.tile_pool(name="ros", bufs=2) as sp,
            tc.tile_pool(name="rop", bufs=2, space="PSUM") as pm_,
            tc.tile_pool(name="rod", bufs=1, space="DRAM") as rdp,
        ):
            # readout
            wmax = cp.tile([64, 16], dt.float32, tag="wmax")
            wmean = cp.tile([64, 16], dt.float32, tag="wmean")
            wsum = cp.tile([64, 16], dt.float32, tag="wsum")
            bout = cp.tile([64, 1], dt.float32, tag="bout")
            gend = cp.tile([64, 1], dt.int32, tag="gend")
            invc = cp.tile([64, 1], dt.float32, tag="invc")
            idro = cp.tile([128, 128], dt.float32, tag="idro")
            nc.sync.dma_start(wmax[:], wmaxd[:])
            nc.sync.dma_start(wmean[:], wmeand[:])
            nc.sync.dma_start(wsum[:], wsumd[:])
            nc.sync.dma_start(bout[:], boutd[:])
            nc.sync.dma_start(gend[:], gendd[:])
            nc.sync.dma_start(invc[:], invcd[:])
            nc.sync.dma_start(idro[:], idf32d[:])
            z16 = cp.tile([16, NLOC], dt.float32, tag="z16")
            r16 = cp.tile([16, NLOC], dt.float32, tag="r16")
            nc.sync.dma_start(z16[:], z16d[:])
            nc.sync.dma_start(r16[:], r16d[:])
            gsumT = cp.tile([16, NLOC], dt.float32, tag="gsumT")
            gmaxT = cp.tile([16, NLOC], dt.float32, tag="gmaxT")
            nc.vector.tensor_tensor_scan(
                out=gsumT[:], data0=z16[:], data1=hmT[:], initial=0.0,
                op0=ALU.mult, op1=ALU.add,
            )
            nc.vector.tensor_tensor_scan(
                out=gmaxT[:], data0=r16[:], data1=hmT[:], initial=-1e30,
                op0=ALU.add, op1=ALU.max,
            )
            gsD = rdp.tile([NLOC, 16], dt.float32, tag="gsD")
            gmD = rdp.tile([NLOC, 16], dt.float32, tag="gmD")
            for w in range(NWIN):
                for (scanT, stage) in ((gsumT, gsD), (gmaxT, gmD)):
                    tp = pm_.tile([128, 16], dt.float32, tag="rops", space="PSUM")
                    nc.tensor.transpose(
                        out=tp[:], in_=scanT[:, w * 128 : (w + 1) * 128],
                        identity=idro[0:16, 0:16],
                    )
                    tsb = sp.tile([128, 16], dt.float32, tag="tsb")
                    nc.vector.tensor_copy(tsb[:], tp[:])
                    nc.sync.dma_start(stage[w * 128 : (w + 1) * 128, :], tsb[:])
            gsE = sp.tile([64, 16], dt.float32, tag="gsE")
            gmE = sp.tile([64, 16], dt.float32, tag="gmE")
            nc.gpsimd.indirect_dma_start(
                out=gsE[:], out_offset=None, in_=gsD[:],
                in_offset=bass.IndirectOffsetOnAxis(ap=gend[:], axis=0),
            )
            nc.gpsimd.indirect_dma_start(
                out=gmE[:], out_offset=None, in_=gmD[:],
                in_offset=bass.IndirectOffsetOnAxis(ap=gend[:], axis=0),
            )
            acc = sp.tile([64, 16], dt.float32, tag="acc")
            tmp2 = sp.tile([64, 16], dt.float32, tag="tmp2")
            # acc = 0.25*gmax*wmax + 0.25*gsum*wsum + 0.25*gsum*invc*wmean
            nc.vector.tensor_mul(acc[:], gmE[:], wmax[:])
            nc.vector.tensor_mul(tmp2[:], gsE[:], wsum[:])
            nc.vector.tensor_add(acc[:], acc[:], tmp2[:])
            nc.vector.tensor_mul(tmp2[:], gsE[:], wmean[:])
            nc.vector.tensor_mul(
                tmp2[:], tmp2[:], invc[:].to_broadcast([64, 16])
            )
            nc.vector.tensor_add(acc[:], acc[:], tmp2[:])
            osum = sp.tile([64, 1], dt.float32, tag="osum")
            nc.vector.tensor_reduce(osum[:], acc[:], axis=AX.X, op=ALU.add)
            o_sb = sp.tile([64, 1], dt.float32, tag="o_sb")
            nc.vector.tensor_scalar_mul(o_sb[:], osum[:], 0.25)
            nc.vector.tensor_add(o_sb[:], o_sb[:], bout[:])
            nc.sync.dma_start(outd[:], o_sb[:])

    split_waits(nc)
    return nc


def make_in_maps(d):
    inputs, gstart = prep(
        np.asarray(d["x"]), np.asarray(d["edge_index"]), np.asarray(d["batch_index"])
    )
    params = prep_params(d)
    import ml_dtypes
    idf32 = np.eye(128, dtype=np.float32)
    idbf = np.eye(128, dtype=ml_dtypes.bfloat16)
    maps = []
    for c in range(NC):
        m = dict(
            xT1=inputs[c]["xT"],
            edata=inputs[c]["edata"],
            z16=inputs[c]["z16"],
            r16=inputs[c]["r16"],
            gends=inputs[c]["gends"],
            invcnt64=inputs[c]["invcnt64"],
            iota=params["iota"],
            idf32=idf32,
            idbf=idbf,
            wmaxr=np.tile(params["Wout"][0:16].reshape(1, 16), (64, 1)),
            wmeanr=np.tile(params["Wout"][16:32].reshape(1, 16), (64, 1)),
            wsumr=np.tile(params["Wout"][32:48].reshape(1, 16), (64, 1)),
            boutr=np.full((64, 1), params["bout"], np.float32),
        )
        for l in (1, 2, 3):
            m[f"W{l}"] = params[f"W{l}"]
            m[f"asrep{l}"] = params[f"asrep{l}"]
            m[f"adrep{l}"] = params[f"adrep{l}"]
            m[f"brep{l}"] = params[f"brep{l}"]
        maps.append(m)
    return maps


_CACHE = {}


def kernel(**inputs) -> np.ndarray:
    d = {k: np.asarray(v) for k, v in inputs.items()}
    maps = make_in_maps(d)
    if "nc" not in _CACHE:
        _CACHE["nc"] = build(NC)
    nc = _CACHE["nc"]
    res = run_bass_kernel_spmd(nc, maps, list(range(NC)))
    got = np.concatenate([res.results[c]["out"].reshape(-1) for c in range(NC)])
    return got.reshape(G, 1).astype(np.float32)



# revision 3
# speedup vs baseline: 3.0361x; 3.0361x over previous
"""Self-contained Trainium2 Bass kernel for the 3-layer GAT + graph readout
(nn_GAT_36361193128013). 8-core SPMD over one trn2 chip.

Structure (v2 — hardware-loop edition):
- graph-aligned node sharding (64 graphs / ~6250 nodes per core) so the
  readout never crosses cores;
- per-layer node table [h bf16(64) | a_s f32(4) | a_d f32(4) | pad] as
  256B rows, AllGather-replicated across the 8 cores;
- all per-window work runs inside tc.For_i hardware loops (50 windows of
  128 dst nodes), with dynamic addressing done exclusively on DRAM via
  DMA DynSlice. This keeps the BIR at a few hundred instructions, which
  makes the per-launch walrus compile (re-run on every invocation under
  the axon/bass2jax path) cheap;
- edge phase per window: T per-tile indirect row gathers by src, one-hot
  select-reduce for the a_d term, exp (no max subtraction), and the
  segment scatter-add as one-hot matmuls accumulating [128 dst, 64ch +
  4 denom] in PSUM;
- layer l+1's dense projection is fused into layer l's node phase
  (transpose -> matmul -> attention dots -> table row write), so only
  layer 1 has a standalone dense loop;
- graph readout via resettable segmented scans + indirect extraction at
  graph boundaries + the final [48->1] projection.

kernel(**inputs) takes the FULL inputs (x, edge_index, batch_index,
weights) and returns the FULL [512, 1] float32 output.
"""
import numpy as np
import ml_dtypes
import concourse.bass as bass
import concourse.mybir as mybir
import concourse.tile as tile
from concourse.bass import ts
from concourse.bass_utils import run_bass_kernel_spmd

dt = mybir.dt
AF = mybir.ActivationFunctionType
ALU = mybir.AluOpType
AX = mybir.AxisListType

H, C = 4, 16
HC = H * C
N = 50000
G = 512
NC = 8
GPC = G // NC          # graphs per core
W = 128                # dst nodes per window
NLOC = 6400            # padded local nodes per core (multiple of 128)
NWIN = NLOC // W       # 50
T = 36                 # gather tiles per window (36*128 = 4608 edge slots)
NTAB = NC * NLOC       # 51200 table rows
PAD_DSTREL = 200.0

# edata int16 cols per window: [srcrow int32 (2T) | dstrel bf16 (T)]
ECOL = 3 * T


def prep(x, edge_index, batch_index, t_tiles):
    """Vectorized host-side prep: per-core node shards + per-window edge
    slot tables. Returns (per-core input dicts, gstart)."""
    src = np.asarray(edge_index[0], dtype=np.int64)
    dst = np.asarray(edge_index[1], dtype=np.int64)
    bi = np.asarray(batch_index, dtype=np.int64)
    x = np.asarray(x)

    gstart = np.searchsorted(bi, np.arange(0, G + 1, GPC))  # node start per core
    core_of_node = np.searchsorted(gstart, np.arange(N), side="right") - 1
    row_of = core_of_node * NLOC + (np.arange(N) - gstart[core_of_node])

    core_of_edge = np.searchsorted(gstart, dst, side="right") - 1
    slots = t_tiles * 128

    inputs = []
    for c in range(NC):
        ns, ne = gstart[c], gstart[c + 1]
        nloc = ne - ns
        m = core_of_edge == c
        e_dst = dst[m] - ns
        e_row = row_of[src[m]]
        # self loops
        e_dst = np.concatenate([e_dst, np.arange(nloc)])
        e_row = np.concatenate([e_row, row_of[ns:ne]])
        order = np.argsort(e_dst, kind="stable")
        e_dst = e_dst[order]
        e_row = e_row[order]

        win = e_dst >> 7
        # slot index within window = rank - start of window
        win_starts = np.searchsorted(win, np.arange(NWIN))
        slot = np.arange(len(e_dst)) - win_starts[win]
        if slot.size and slot.max() >= slots:
            raise OverflowError(int(slot.max()) + 1)
        sr = np.zeros((NWIN, slots), np.int32)          # pad -> row 0
        drl = np.full((NWIN, slots), PAD_DSTREL, np.float32)
        sr[win, slot] = e_row
        drl[win, slot] = e_dst - (win << 7)

        # slot k = t*128 + p lands at [p, t]
        srp = sr.reshape(NWIN, t_tiles, 128).transpose(0, 2, 1)    # [NWIN,128,T]
        drp = drl.reshape(NWIN, t_tiles, 128).transpose(0, 2, 1)
        esrc = np.ascontiguousarray(srp.transpose(1, 0, 2)).view(np.int16).reshape(
            128, NWIN * 2 * t_tiles
        )
        edrl = (
            np.ascontiguousarray(drp.transpose(1, 0, 2))
            .astype(ml_dtypes.bfloat16)
            .reshape(128, NWIN * t_tiles)
        )

        xT = np.zeros((128, NLOC), ml_dtypes.bfloat16)
        xT[:, :nloc] = x[ns:ne].T.astype(ml_dtypes.bfloat16)

        # readout: graph boundaries within the core
        bounds = np.searchsorted(bi, np.arange(c * GPC, (c + 1) * GPC + 1)) - ns
        z = np.ones(NLOC, np.float32)
        r = np.zeros(NLOC, np.float32)
        z[bounds[:-1]] = 0.0
        r[bounds[:-1]] = -1e30
        z16 = np.tile(z, (16, 1)).astype(ml_dtypes.bfloat16)
        r16 = np.tile(r, (16, 1)).astype(ml_dtypes.bfloat16)
        gends = (bounds[1:] - 1).astype(np.int32).reshape(GPC, 1)
        cnt = np.diff(bounds).astype(np.float32)
        invcnt64 = (1.0 / np.maximum(cnt, 1.0)).astype(np.float32).reshape(GPC, 1)

        inputs.append(
            dict(xT1=xT, esrc=esrc, edrl=edrl, z16=z16, r16=r16, gends=gends, invcnt64=invcnt64)
        )
    return inputs, gstart


def null_input_decls():
    """Inputs the timing-floor null kernel should also upload (largest bufs)."""
    return [
        ("xT1", [128, NLOC], dt.bfloat16),
        ("esrc", [128, NWIN * 2 * T], dt.int16),
        ("edrl", [128, NWIN * T], dt.bfloat16),
    ]


_ctr = [0]


def split_waits(nc):
    """Walrus codegen only supports one wait per instruction; split extras
    onto NoOps."""
    for _name, bbwrap in nc.bb_map.items():
        bb = bbwrap.bb if hasattr(bbwrap, "bb") else bbwrap
        insts = bb.instructions
        i = 0
        while i < len(insts):
            inst = insts[i]
            si = inst.sync_info
            if si is not None and si.on_wait and len(si.on_wait) > 1:
                waits = list(si.on_wait)
                si.on_wait = waits[:1]
                rest = waits[1:]
                for w in rest:
                    _ctr[0] += 1
                    nop = mybir.InstNoOp(name=f"splitw-{_ctr[0]}", ins=[], outs=[])
                    nop.engine = inst.engine
                    nop.sync_info = mybir.SyncInfo(on_wait=[w], on_update=[])
                    nc.register_instruction(nop)
                    insts.insert(i, nop)
                    i += 1
            i += 1


def build(n_cores=8, t_tiles=T):
    TT = t_tiles
    ecol = 3 * TT
    nc = bass.Bass(target_bir_lowering=False)

    xT1 = nc.declare_dram_parameter("xT1", [128, NLOC], dt.bfloat16, isOutput=False)
    esrcd = nc.declare_dram_parameter("esrc", [128, NWIN * 2 * TT], dt.int16, isOutput=False)
    edrld = nc.declare_dram_parameter("edrl", [128, NWIN * TT], dt.bfloat16, isOutput=False)
    z16d = nc.declare_dram_parameter("z16", [16, NLOC], dt.bfloat16, isOutput=False)
    r16d = nc.declare_dram_parameter("r16", [16, NLOC], dt.bfloat16, isOutput=False)
    gendd = nc.declare_dram_parameter("gends", [64, 1], dt.int32, isOutput=False)
    invcd = nc.declare_dram_parameter("invcnt64", [64, 1], dt.float32, isOutput=False)
    Wd = {
        1: nc.declare_dram_parameter("W1", [128, 64], dt.bfloat16, isOutput=False),
        2: nc.declare_dram_parameter("W2", [64, 64], dt.bfloat16, isOutput=False),
        3: nc.declare_dram_parameter("W3", [64, 64], dt.bfloat16, isOutput=False),
    }
    asd, add_, bd = {}, {}, {}
    for l in (1, 2, 3):
        asd[l] = nc.declare_dram_parameter(f"asrep{l}", [128, 64], dt.float32, isOutput=False)
        add_[l] = nc.declare_dram_parameter(f"adrep{l}", [128, 64], dt.float32, isOutput=False)
        bd[l] = nc.declare_dram_parameter(f"brep{l}", [128, 64], dt.float32, isOutput=False)
    iotad = nc.declare_dram_parameter("iota", [128, 128], dt.bfloat16, isOutput=False)
    idf32d = nc.declare_dram_parameter("idf32", [128, 128], dt.float32, isOutput=False)
    wmaxd = nc.declare_dram_parameter("wmaxr", [64, 16], dt.float32, isOutput=False)
    wmeand = nc.declare_dram_parameter("wmeanr", [64, 16], dt.float32, isOutput=False)
    wsumd = nc.declare_dram_parameter("wsumr", [64, 16], dt.float32, isOutput=False)
    boutd = nc.declare_dram_parameter("boutr", [64, 1], dt.float32, isOutput=False)
    outd = nc.declare_dram_parameter("out", [64, 1], dt.float32, isOutput=True)

    with tile.TileContext(nc) as tc:
      with tc.tile_pool(name="dram", bufs=1, space="DRAM") as dp:
        # DRAM internals (outlive both phases)
        myrows = dp.tile([NLOC, 128], dt.bfloat16, tag="myrows")
        table = dp.tile([NTAB, 128], dt.bfloat16, tag="table")
        adtab = dp.tile([4, NLOC], dt.bfloat16, tag="adtab")
        hmD = dp.tile([16, NLOC], dt.float32, tag="hmD")
        hstg = dp.tile([128, NWIN * t_tiles * 128], dt.bfloat16, tag="hstg")
        gsD = dp.tile([NLOC, 16], dt.float32, tag="gsD")
        gmD = dp.tile([NLOC, 16], dt.float32, tag="gmD")
        with (
            tc.tile_pool(name="const", bufs=1) as cp,
            tc.tile_pool(name="work", bufs=2) as wp,
            tc.tile_pool(name="sml", bufs=3) as sp,
            tc.tile_pool(name="ps_acc", bufs=2, space="PSUM") as pa,
            tc.tile_pool(name="ps_misc", bufs=2, space="PSUM") as pm_,
        ):
            def ld(dram, shape, dtp):
                t = cp.tile(shape, dtp, tag=dram.name)
                nc.sync.dma_start(t[:], dram[:])
                return t

            Ws = {l: ld(Wd[l], Wd[l].shape, dt.bfloat16) for l in (1, 2, 3)}
            ass = {l: ld(asd[l], [128, 64], dt.float32) for l in (1, 2, 3)}
            ads = {l: ld(add_[l], [128, 64], dt.float32) for l in (1, 2, 3)}
            bs = {l: ld(bd[l], [128, 64], dt.float32) for l in (1, 2, 3)}
            iota = ld(iotad, [128, 128], dt.bfloat16)
            idf32 = ld(idf32d, [128, 128], dt.float32)

            def table_row_write(iv, l, h_sb):
                """From h [128,64] f32 (SBUF): attention dots, table row pack,
                myrows + adtab writes for layer l at window iv."""
                tmp = sp.tile([128, 64], dt.float32, tag="astmp")
                asad = sp.tile([128, 8], dt.float32, tag="asad")
                nc.vector.tensor_mul(tmp[:], h_sb[:], ass[l][:])
                nc.vector.tensor_reduce(
                    asad[:, 0:4], tmp[:].rearrange("p (h c) -> p h c", c=16),
                    axis=AX.X, op=ALU.add,
                )
                nc.vector.tensor_mul(tmp[:], h_sb[:], ads[l][:])
                nc.vector.tensor_reduce(
                    asad[:, 4:8], tmp[:].rearrange("p (h c) -> p h c", c=16),
                    axis=AX.X, op=ALU.add,
                )
                rowt = sp.tile([128, 128], dt.bfloat16, tag="rowt")
                nc.vector.tensor_copy(rowt[:, 0:64], h_sb[:])
                nc.vector.tensor_copy(rowt[:, 64:80].bitcast(dt.float32), asad[:])
                nc.sync.dma_start(myrows[ts(iv, 128), :], rowt[:])
                adt_ps = pm_.tile([4, 128], dt.float32, tag="adt_ps", space="PSUM")
                nc.tensor.transpose(out=adt_ps[:], in_=asad[:, 4:8], identity=idf32[:])
                ad_bf = sp.tile([4, 128], dt.bfloat16, tag="ad_bf")
                nc.vector.tensor_copy(ad_bf[:], adt_ps[:])
                nc.sync.dma_start(adtab[:, ts(iv, 128)], ad_bf[:])

            def allgather():
                nc.gpsimd.collective_compute(
                    "AllGather",
                    ALU.bypass,
                    replica_groups=[list(range(n_cores))],
                    ins=[myrows[:].opt()],
                    outs=[table[:].opt()],
                )

            # src-row indices for every window, resident in SBUF for the
            # (static) gather streams of all three layers
            esrc_sb = cp.tile([128, NWIN * 2 * TT], dt.int16, tag="esrc_sb")
            nc.sync.dma_start(esrc_sb[:], esrcd[:])

            def gather_phase():
                # statically unrolled indirect gathers (walrus cannot codegen
                # indirect DMA inside hardware loops): table rows by src for
                # every window, staged to DRAM for the compute loop
                for w in range(NWIN):
                    hsg = wp.tile([128, TT, 128], dt.bfloat16, tag="hsg")
                    srw = esrc_sb[:, w * 2 * TT : (w + 1) * 2 * TT].bitcast(dt.int32)
                    for t in range(TT):
                        nc.gpsimd.indirect_dma_start(
                            out=hsg[:, t, :], out_offset=None, in_=table[:],
                            in_offset=bass.IndirectOffsetOnAxis(
                                ap=srw[:, t : t + 1], axis=0
                            ),
                        )
                    nc.sync.dma_start(
                        hstg[:, w * TT * 128 : (w + 1) * TT * 128],
                        hsg[:].rearrange("p t c -> p (t c)"),
                    )

            # ---- dense phase, layer 1 ----
            with tc.For_i(0, NWIN, 1) as iv:
                xc = wp.tile([128, 128], dt.bfloat16, tag="xc")
                nc.sync.dma_start(xc[:], xT1[:, ts(iv, 128)])
                h_ps = pm_.tile([128, 64], dt.float32, tag="h_ps", space="PSUM")
                nc.tensor.matmul(out=h_ps[:], lhsT=xc[:], rhs=Ws[1][:], start=True, stop=True)
                h_sb = sp.tile([128, 64], dt.float32, tag="h_sb")
                nc.vector.tensor_copy(h_sb[:], h_ps[:])
                table_row_write(iv, 1, h_sb)
            allgather()

            # ---- edge phase per layer (layer l+1 dense fused in) ----
            for l in (1, 2, 3):
                gather_phase()
                with tc.For_i(0, NWIN, 1) as iv:
                    drel = wp.tile([128, TT], dt.bfloat16, tag="drel")
                    nc.sync.dma_start(drel[:], edrld[:, ts(iv, TT)])
                    hsrc = wp.tile([128, TT, 128], dt.bfloat16, tag="hsrc")
                    nc.sync.dma_start(
                        hsrc[:].rearrange("p t c -> p (t c)"),
                        hstg[:, ts(iv, TT * 128)],
                    )
                    S3 = wp.tile([128, TT, W], dt.bfloat16, tag="S3")
                    nc.vector.tensor_tensor(
                        out=S3[:],
                        in0=drel[:].to_broadcast([128, TT, W]),
                        in1=iota[:].unsqueeze(1).to_broadcast([128, TT, W]),
                        op=ALU.is_equal,
                    )
                    adrep = wp.tile([128, 4, 128], dt.bfloat16, tag="adrep")
                    nc.sync.dma_start(
                        adrep[:],
                        adtab[:, ts(iv, 128)].unsqueeze(0).to_broadcast([128, 4, 128]),
                    )
                    ade = wp.tile([128, TT, 4], dt.float32, tag="ade")
                    tmph = wp.tile([128, TT, 2, 128], dt.bfloat16, tag="tmph")
                    for h in range(0, 4, 2):
                        nc.vector.tensor_tensor(
                            out=tmph[:],
                            in0=S3[:].unsqueeze(2).to_broadcast([128, TT, 2, 128]),
                            in1=adrep[:, h : h + 2, :].unsqueeze(1).to_broadcast(
                                [128, TT, 2, 128]
                            ),
                            op=ALU.mult,
                        )
                        nc.vector.tensor_reduce(
                            ade[:, :, h : h + 2], tmph[:], axis=AX.X, op=ALU.add,
                        )
                    e_sb = sp.tile([128, TT * 4], dt.float32, tag="e_sb")
                    nc.vector.tensor_tensor(
                        out=e_sb[:].rearrange("p (t f) -> p t f", f=4),
                        in0=hsrc[:, :, 64:72].bitcast(dt.float32),
                        in1=ade[:],
                        op=ALU.add,
                    )
                    nc.vector.scalar_tensor_tensor(
                        out=e_sb[:], in0=e_sb[:], scalar=0.2, in1=e_sb[:],
                        op0=ALU.mult, op1=ALU.max,
                    )
                    wmsg = wp.tile([128, TT, 72], dt.bfloat16, tag="wmsg")
                    nc.scalar.activation(
                        wmsg[:, :, 64:68], e_sb[:].rearrange("p (t f) -> p t f", f=4),
                        AF.Exp,
                    )
                    nc.vector.tensor_tensor(
                        out=wmsg[:, :, 0:64].rearrange("p t (h c) -> p t h c", c=16),
                        in0=hsrc[:, :, 0:64].rearrange("p t (h c) -> p t h c", c=16),
                        in1=wmsg[:, :, 64:68].unsqueeze(3).to_broadcast([128, TT, 4, 16]),
                        op=ALU.mult,
                    )
                    out_ps = pa.tile([128, 68], dt.float32, tag="out_ps", space="PSUM")
                    for t in range(TT):
                        nc.tensor.matmul(
                            out=out_ps[:],
                            lhsT=S3[:, t, :],
                            rhs=wmsg[:, t, 0:68],
                            start=(t == 0), stop=(t == TT - 1),
                        )
                    # node phase
                    sg = sp.tile([128, 4], dt.float32, tag="sg")
                    nc.vector.tensor_scalar_add(sg[:], out_ps[:, 64:68], 1e-30)
                    rs = sp.tile([128, 4], dt.float32, tag="rs")
                    nc.vector.reciprocal(rs[:], sg[:])
                    xn = sp.tile([128, 64], dt.float32, tag="xn")
                    nc.vector.tensor_tensor(
                        out=xn[:].rearrange("p (h c) -> p h c", c=16),
                        in0=out_ps[:, 0:64].rearrange("p (h c) -> p h c", c=16),
                        in1=rs[:].unsqueeze(2).to_broadcast([128, 4, 16]),
                        op=ALU.mult,
                    )
                    nc.vector.tensor_add(xn[:], xn[:], bs[l][:])
                    nc.scalar.activation(xn[:], xn[:], AF.Tanh)
                    if l < 3:
                        # fused dense for layer l+1
                        xt_ps = pm_.tile([64, 128], dt.float32, tag="xt_ps", space="PSUM")
                        nc.tensor.transpose(out=xt_ps[:], in_=xn[:], identity=idf32[:])
                        xt_sb = sp.tile([64, 128], dt.bfloat16, tag="xt_sb")
                        nc.vector.tensor_copy(xt_sb[:], xt_ps[:])
                        h2_ps = pm_.tile([128, 64], dt.float32, tag="h_ps", space="PSUM")
                        nc.tensor.matmul(
                            out=h2_ps[:], lhsT=xt_sb[:], rhs=Ws[l + 1][:],
                            start=True, stop=True,
                        )
                        h2_sb = sp.tile([128, 64], dt.float32, tag="h_sb")
                        nc.vector.tensor_copy(h2_sb[:], h2_ps[:])
                        table_row_write(iv, l + 1, h2_sb)
                    else:
                        hm = sp.tile([128, 16], dt.float32, tag="hm")
                        nc.vector.tensor_reduce(
                            hm[:], xn[:].rearrange("p (h c) -> p c h", c=16),
                            axis=AX.X, op=ALU.add,
                        )
                        hm_ps = pm_.tile([16, 128], dt.float32, tag="xt_ps", space="PSUM")
                        nc.tensor.transpose(out=hm_ps[:], in_=hm[:], identity=idf32[:])
                        hm_sb = sp.tile([16, 128], dt.float32, tag="hm_sb")
                        nc.vector.tensor_copy(hm_sb[:], hm_ps[:])
                        nc.sync.dma_start(hmD[:, ts(iv, 128)], hm_sb[:])
                if l < 3:
                    allgather()

        # ---- readout (own pool scope; SBUF from the layer phase is freed) ----
        with (
            tc.tile_pool(name="ro", bufs=1) as cp,
            tc.tile_pool(name="ros", bufs=2) as sp,
            tc.tile_pool(name="rop", bufs=2, space="PSUM") as pm_,
        ):
            idro = cp.tile([16, 16], dt.float32, tag="idro")
            nc.sync.dma_start(idro[:], idf32d[0:16, 0:16])
            hmT = cp.tile([16, NLOC], dt.float32, tag="hmT")
            nc.sync.dma_start(hmT[:], hmD[:])
            z16 = cp.tile([16, NLOC], dt.bfloat16, tag="z16")
            r16 = cp.tile([16, NLOC], dt.bfloat16, tag="r16")
            nc.sync.dma_start(z16[:], z16d[:])
            nc.sync.dma_start(r16[:], r16d[:])
            gsumT = cp.tile([16, NLOC], dt.float32, tag="gsumT")
            gmaxT = cp.tile([16, NLOC], dt.float32, tag="gmaxT")
            nc.vector.tensor_tensor_scan(
                out=gsumT[:], data0=z16[:], data1=hmT[:], initial=0.0,
                op0=ALU.mult, op1=ALU.add,
            )
            nc.vector.tensor_tensor_scan(
                out=gmaxT[:], data0=r16[:], data1=hmT[:], initial=-1e30,
                op0=ALU.add, op1=ALU.max,
            )
            for w in range(NWIN):
                for (scanT, stage, tg) in ((gsumT, gsD, "s"), (gmaxT, gmD, "m")):
                    tp = pm_.tile([128, 16], dt.float32, tag="rops" + tg, space="PSUM")
                    nc.tensor.transpose(
                        out=tp[:], in_=scanT[:, w * 128 : (w + 1) * 128],
                        identity=idro[:],
                    )
                    tsb = sp.tile([128, 16], dt.float32, tag="tsb" + tg)
                    nc.vector.tensor_copy(tsb[:], tp[:])
                    nc.sync.dma_start(stage[w * 128 : (w + 1) * 128, :], tsb[:])

            wmax = cp.tile([64, 16], dt.float32, tag="wmax")
            wmean = cp.tile([64, 16], dt.float32, tag="wmean")
            wsum = cp.tile([64, 16], dt.float32, tag="wsum")
            bout = cp.tile([64, 1], dt.float32, tag="bout")
            gend = cp.tile([64, 1], dt.int32, tag="gend")
            invc = cp.tile([64, 1], dt.float32, tag="invc")
            nc.sync.dma_start(wmax[:], wmaxd[:])
            nc.sync.dma_start(wmean[:], wmeand[:])
            nc.sync.dma_start(wsum[:], wsumd[:])
            nc.sync.dma_start(bout[:], boutd[:])
            nc.sync.dma_start(gend[:], gendd[:])
            nc.sync.dma_start(invc[:], invcd[:])
            gsE = sp.tile([64, 16], dt.float32, tag="gsE")
            gmE = sp.tile([64, 16], dt.float32, tag="gmE")
            nc.gpsimd.indirect_dma_start(
                out=gsE[:], out_offset=None, in_=gsD[:],
                in_offset=bass.IndirectOffsetOnAxis(ap=gend[:], axis=0),
            )
            nc.gpsimd.indirect_dma_start(
                out=gmE[:], out_offset=None, in_=gmD[:],
                in_offset=bass.IndirectOffsetOnAxis(ap=gend[:], axis=0),
            )
            acc = sp.tile([64, 16], dt.float32, tag="acc")
            tmp2 = sp.tile([64, 16], dt.float32, tag="tmp2")
            # acc = gmax*wmax + gsum*wsum + gsum*invc*wmean  (x0.25 at the end)
            nc.vector.tensor_mul(acc[:], gmE[:], wmax[:])
            nc.vector.tensor_mul(tmp2[:], gsE[:], wsum[:])
            nc.vector.tensor_add(acc[:], acc[:], tmp2[:])
            nc.vector.tensor_mul(tmp2[:], gsE[:], wmean[:])
            nc.vector.tensor_mul(tmp2[:], tmp2[:], invc[:].to_broadcast([64, 16]))
            nc.vector.tensor_add(acc[:], acc[:], tmp2[:])
            osum = sp.tile([64, 1], dt.float32, tag="osum")
            nc.vector.tensor_reduce(osum[:], acc[:], axis=AX.X, op=ALU.add)
            o_sb = sp.tile([64, 1], dt.float32, tag="o_sb")
            nc.vector.tensor_scalar_mul(o_sb[:], osum[:], 0.25)
            nc.vector.tensor_add(o_sb[:], o_sb[:], bout[:])
            nc.sync.dma_start(outd[:], o_sb[:])

    split_waits(nc)
    return nc


def prep_params(d):
    """Replicated parameter tensors (same for all cores)."""
    out = {}
    out["iota"] = np.tile(np.arange(W, dtype=np.float32), (128, 1)).astype(
        ml_dtypes.bfloat16
    )
    out["idf32"] = np.eye(128, dtype=np.float32)
    for l, fin in ((1, 128), (2, HC), (3, HC)):
        Wl = np.asarray(d[f"W{l}"], np.float32)
        out[f"W{l}"] = Wl.astype(ml_dtypes.bfloat16)
        out[f"asrep{l}"] = np.tile(
            np.asarray(d[f"as{l}"], np.float32).reshape(1, HC), (128, 1)
        )
        out[f"adrep{l}"] = np.tile(
            np.asarray(d[f"ad{l}"], np.float32).reshape(1, HC), (128, 1)
        )
        out[f"brep{l}"] = np.tile(
            np.asarray(d[f"b{l}"], np.float32).reshape(1, HC), (128, 1)
        )
    Wout = np.asarray(d["Wout"], np.float32)
    out["wmaxr"] = np.tile(Wout[0:16].reshape(1, 16), (64, 1))
    out["wmeanr"] = np.tile(Wout[16:32].reshape(1, 16), (64, 1))
    out["wsumr"] = np.tile(Wout[32:48].reshape(1, 16), (64, 1))
    out["boutr"] = np.full((64, 1), np.float32(np.asarray(d["bout"]).reshape(-1)[0]))
    return out


def make_in_maps(d, t_tiles=T):
    inputs, _ = prep(d["x"], d["edge_index"], d["batch_index"], t_tiles)
    params = prep_params(d)
    maps = []
    for c in range(NC):
        m = dict(inputs[c])
        m.update(params)
        maps.append(m)
    return maps


_CACHE = {}


def kernel(**inputs) -> np.ndarray:
    d = {k: np.asarray(v) for k, v in inputs.items()}
    t_tiles = T
    while True:
        try:
            maps = make_in_maps(d, t_tiles)
            break
        except OverflowError as e:
            t_tiles = -(-int(e.args[0]) // 128)
    if t_tiles not in _CACHE:
        _CACHE[t_tiles] = build(NC, t_tiles)
    nc = _CACHE[t_tiles]
    res = run_bass_kernel_spmd(nc, maps, list(range(NC)))
    got = np.concatenate([res.results[c]["out"].reshape(-1) for c in range(NC)])
    return got.reshape(G, 1).astype(np.float32)


# revision 4
# speedup vs baseline: 3.2900x; 1.0836x over previous
"""Self-contained Trainium2 Bass kernel for the 3-layer GAT + graph readout
(nn_GAT_36361193128013). 8-core SPMD over one trn2 chip.

Structure (v2 — hardware-loop edition):
- graph-aligned node sharding (64 graphs / ~6250 nodes per core) so the
  readout never crosses cores;
- per-layer node table [h bf16(64) | a_s f32(4) | a_d f32(4) | pad] as
  256B rows, AllGather-replicated across the 8 cores;
- all per-window work runs inside tc.For_i hardware loops (50 windows of
  128 dst nodes), with dynamic addressing done exclusively on DRAM via
  DMA DynSlice. This keeps the BIR at a few hundred instructions, which
  makes the per-launch walrus compile (re-run on every invocation under
  the axon/bass2jax path) cheap;
- edge phase per window: T per-tile indirect row gathers by src, one-hot
  select-reduce for the a_d term, exp (no max subtraction), and the
  segment scatter-add as one-hot matmuls accumulating [128 dst, 64ch +
  4 denom] in PSUM;
- layer l+1's dense projection is fused into layer l's node phase
  (transpose -> matmul -> attention dots -> table row write), so only
  layer 1 has a standalone dense loop;
- graph readout via resettable segmented scans + indirect extraction at
  graph boundaries + the final [48->1] projection.

kernel(**inputs) takes the FULL inputs (x, edge_index, batch_index,
weights) and returns the FULL [512, 1] float32 output.
"""
import numpy as np
import ml_dtypes
import concourse.bass as bass
import concourse.mybir as mybir
import concourse.tile as tile
from concourse.bass import ts
from concourse.bass_utils import run_bass_kernel_spmd

dt = mybir.dt
AF = mybir.ActivationFunctionType
ALU = mybir.AluOpType
AX = mybir.AxisListType

H, C = 4, 16
HC = H * C
N = 50000
G = 512
NC = 8
GPC = G // NC          # graphs per core
W = 128                # dst nodes per window
NLOC = 6400            # padded local nodes per core (multiple of 128)
NWIN = NLOC // W       # 50
T = 35                 # gather tiles per window (35*128 = 4480 edge slots)
NTAB = NC * NLOC       # 51200 table rows
PAD_DSTREL = 200.0

# edata int16 cols per window: [srcrow int32 (2T) | dstrel bf16 (T)]
ECOL = 3 * T


def prep(x, edge_index, batch_index, t_tiles):
    """Vectorized host-side prep: per-core node shards + per-window edge
    slot tables. Returns (per-core input dicts, gstart)."""
    src = np.asarray(edge_index[0], dtype=np.int64)
    dst = np.asarray(edge_index[1], dtype=np.int64)
    bi = np.asarray(batch_index, dtype=np.int64)
    x = np.asarray(x)

    gstart = np.searchsorted(bi, np.arange(0, G + 1, GPC))  # node start per core
    core_of_node = np.searchsorted(gstart, np.arange(N), side="right") - 1
    row_of = core_of_node * NLOC + (np.arange(N) - gstart[core_of_node])

    core_of_edge = np.searchsorted(gstart, dst, side="right") - 1
    slots = t_tiles * 128

    inputs = []
    for c in range(NC):
        ns, ne = gstart[c], gstart[c + 1]
        nloc = ne - ns
        m = core_of_edge == c
        e_dst = dst[m] - ns
        e_row = row_of[src[m]]
        # self loops
        e_dst = np.concatenate([e_dst, np.arange(nloc)])
        e_row = np.concatenate([e_row, row_of[ns:ne]])
        order = np.argsort(e_dst, kind="stable")
        e_dst = e_dst[order]
        e_row = e_row[order]

        win = e_dst >> 7
        # slot index within window = rank - start of window
        win_starts = np.searchsorted(win, np.arange(NWIN))
        slot = np.arange(len(e_dst)) - win_starts[win]
        if slot.size and slot.max() >= slots:
            raise OverflowError(int(slot.max()) + 1)
        sr = np.zeros((NWIN, slots), np.int32)          # pad -> row 0
        drl = np.full((NWIN, slots), PAD_DSTREL, np.float32)
        sr[win, slot] = e_row
        drl[win, slot] = e_dst - (win << 7)

        # slot k = t*128 + p lands at [p, t]
        srp = sr.reshape(NWIN, t_tiles, 128).transpose(0, 2, 1)    # [NWIN,128,T]
        drp = drl.reshape(NWIN, t_tiles, 128).transpose(0, 2, 1)
        esrc = np.ascontiguousarray(srp.transpose(1, 0, 2)).view(np.int16).reshape(
            128, NWIN * 2 * t_tiles
        )
        edrl = (
            np.ascontiguousarray(drp.transpose(1, 0, 2))
            .astype(ml_dtypes.bfloat16)
            .reshape(128, NWIN * t_tiles)
        )

        xT = np.zeros((128, NLOC), ml_dtypes.bfloat16)
        xT[:, :nloc] = x[ns:ne].T.astype(ml_dtypes.bfloat16)

        # readout: graph boundaries within the core
        bounds = np.searchsorted(bi, np.arange(c * GPC, (c + 1) * GPC + 1)) - ns
        z = np.ones(NLOC, np.float32)
        r = np.zeros(NLOC, np.float32)
        z[bounds[:-1]] = 0.0
        r[bounds[:-1]] = -1e30
        z16 = z.reshape(1, NLOC).astype(ml_dtypes.bfloat16)
        r16 = r.reshape(1, NLOC).astype(ml_dtypes.bfloat16)
        gends = (bounds[1:] - 1).astype(np.int32).reshape(GPC, 1)
        cnt = np.diff(bounds).astype(np.float32)
        invcnt64 = (1.0 / np.maximum(cnt, 1.0)).astype(np.float32).reshape(GPC, 1)

        inputs.append(
            dict(xT1=xT, esrc=esrc, edrl=edrl, z16=z16, r16=r16, gends=gends, invcnt64=invcnt64)
        )
    return inputs, gstart


def null_input_decls():
    """Inputs the timing-floor null kernel should also upload (largest bufs)."""
    return [
        ("xT1", [128, NLOC], dt.bfloat16),
        ("esrc", [128, NWIN * 2 * T], dt.int16),
        ("edrl", [128, NWIN * T], dt.bfloat16),
    ]


_ctr = [0]


def split_waits(nc):
    """Walrus codegen only supports one wait per instruction; split extras
    onto NoOps."""
    for _name, bbwrap in nc.bb_map.items():
        bb = bbwrap.bb if hasattr(bbwrap, "bb") else bbwrap
        insts = bb.instructions
        i = 0
        while i < len(insts):
            inst = insts[i]
            si = inst.sync_info
            if si is not None and si.on_wait and len(si.on_wait) > 1:
                waits = list(si.on_wait)
                si.on_wait = waits[:1]
                rest = waits[1:]
                for w in rest:
                    _ctr[0] += 1
                    nop = mybir.InstNoOp(name=f"splitw-{_ctr[0]}", ins=[], outs=[])
                    nop.engine = inst.engine
                    nop.sync_info = mybir.SyncInfo(on_wait=[w], on_update=[])
                    nc.register_instruction(nop)
                    insts.insert(i, nop)
                    i += 1
            i += 1


def build(n_cores=8, t_tiles=T):
    TT = t_tiles
    ecol = 3 * TT
    nc = bass.Bass(target_bir_lowering=False)

    xT1 = nc.declare_dram_parameter("xT1", [128, NLOC], dt.bfloat16, isOutput=False)
    esrcd = nc.declare_dram_parameter("esrc", [128, NWIN * 2 * TT], dt.int16, isOutput=False)
    edrld = nc.declare_dram_parameter("edrl", [128, NWIN * TT], dt.bfloat16, isOutput=False)
    z16d = nc.declare_dram_parameter("z16", [1, NLOC], dt.bfloat16, isOutput=False)
    r16d = nc.declare_dram_parameter("r16", [1, NLOC], dt.bfloat16, isOutput=False)
    gendd = nc.declare_dram_parameter("gends", [64, 1], dt.int32, isOutput=False)
    invcd = nc.declare_dram_parameter("invcnt64", [64, 1], dt.float32, isOutput=False)
    Wd = {
        1: nc.declare_dram_parameter("W1", [128, 64], dt.bfloat16, isOutput=False),
        2: nc.declare_dram_parameter("W2", [64, 64], dt.bfloat16, isOutput=False),
        3: nc.declare_dram_parameter("W3", [64, 64], dt.bfloat16, isOutput=False),
    }
    asd, add_, bd = {}, {}, {}
    for l in (1, 2, 3):
        asd[l] = nc.declare_dram_parameter(f"asrep{l}", [1, 64], dt.float32, isOutput=False)
        add_[l] = nc.declare_dram_parameter(f"adrep{l}", [1, 64], dt.float32, isOutput=False)
        bd[l] = nc.declare_dram_parameter(f"brep{l}", [1, 64], dt.float32, isOutput=False)
    iotad = nc.declare_dram_parameter("iota", [1, 128], dt.bfloat16, isOutput=False)
    idf32d = nc.declare_dram_parameter("idf32", [128, 128], dt.float32, isOutput=False)
    wmaxd = nc.declare_dram_parameter("wmaxr", [64, 16], dt.float32, isOutput=False)
    wmeand = nc.declare_dram_parameter("wmeanr", [64, 16], dt.float32, isOutput=False)
    wsumd = nc.declare_dram_parameter("wsumr", [64, 16], dt.float32, isOutput=False)
    boutd = nc.declare_dram_parameter("boutr", [64, 1], dt.float32, isOutput=False)
    outd = nc.declare_dram_parameter("out", [64, 1], dt.float32, isOutput=True)

    with tile.TileContext(nc) as tc:
      with tc.tile_pool(name="dram", bufs=1, space="DRAM") as dp:
        # DRAM internals (outlive both phases)
        myrows = dp.tile([NLOC, 128], dt.bfloat16, tag="myrows")
        table = dp.tile([NTAB, 128], dt.bfloat16, tag="table")
        adtab = dp.tile([4, NLOC], dt.bfloat16, tag="adtab")
        hmD = dp.tile([16, NLOC], dt.float32, tag="hmD")
        hstg = dp.tile([128, NWIN * t_tiles * 128], dt.bfloat16, tag="hstg")
        gsD = dp.tile([NLOC, 16], dt.float32, tag="gsD")
        gmD = dp.tile([NLOC, 16], dt.float32, tag="gmD")
        with (
            tc.tile_pool(name="const", bufs=1) as cp,
            tc.tile_pool(name="work", bufs=2) as wp,
            tc.tile_pool(name="sml", bufs=3) as sp,
            tc.tile_pool(name="ps_acc", bufs=2, space="PSUM") as pa,
            tc.tile_pool(name="ps_misc", bufs=2, space="PSUM") as pm_,
        ):
            def ld(dram, shape, dtp):
                t = cp.tile(shape, dtp, tag=dram.name)
                nc.sync.dma_start(t[:], dram[:])
                return t

            def ldb(dram, shape, dtp):
                # single-row param broadcast to all partitions at load time
                t = cp.tile(shape, dtp, tag=dram.name)
                nc.sync.dma_start(t[:], dram[0:1, :].to_broadcast(shape))
                return t

            Ws = {l: ld(Wd[l], Wd[l].shape, dt.bfloat16) for l in (1, 2, 3)}
            ass = {l: ldb(asd[l], [128, 64], dt.float32) for l in (1, 2, 3)}
            ads = {l: ldb(add_[l], [128, 64], dt.float32) for l in (1, 2, 3)}
            bs = {l: ldb(bd[l], [128, 64], dt.float32) for l in (1, 2, 3)}
            iota = ldb(iotad, [128, 128], dt.bfloat16)
            idf32 = ld(idf32d, [128, 128], dt.float32)

            def table_row_write(iv, l, h_sb):
                """From h [128,64] f32 (SBUF): attention dots, table row pack,
                myrows + adtab writes for layer l at window iv."""
                tmp = sp.tile([128, 64], dt.float32, tag="astmp")
                asad = sp.tile([128, 8], dt.float32, tag="asad")
                nc.vector.tensor_mul(tmp[:], h_sb[:], ass[l][:])
                nc.vector.tensor_reduce(
                    asad[:, 0:4], tmp[:].rearrange("p (h c) -> p h c", c=16),
                    axis=AX.X, op=ALU.add,
                )
                nc.vector.tensor_mul(tmp[:], h_sb[:], ads[l][:])
                nc.vector.tensor_reduce(
                    asad[:, 4:8], tmp[:].rearrange("p (h c) -> p h c", c=16),
                    axis=AX.X, op=ALU.add,
                )
                rowt = sp.tile([128, 128], dt.bfloat16, tag="rowt")
                nc.vector.tensor_copy(rowt[:, 0:64], h_sb[:])
                nc.vector.tensor_copy(rowt[:, 64:80].bitcast(dt.float32), asad[:])
                nc.sync.dma_start(myrows[ts(iv, 128), :], rowt[:])
                adt_ps = pm_.tile([4, 128], dt.float32, tag="adt_ps", space="PSUM")
                nc.tensor.transpose(out=adt_ps[:], in_=asad[:, 4:8], identity=idf32[:])
                ad_bf = sp.tile([4, 128], dt.bfloat16, tag="ad_bf")
                nc.vector.tensor_copy(ad_bf[:], adt_ps[:])
                nc.sync.dma_start(adtab[:, ts(iv, 128)], ad_bf[:])

            def allgather():
                nc.gpsimd.collective_compute(
                    "AllGather",
                    ALU.bypass,
                    replica_groups=[list(range(n_cores))],
                    ins=[myrows[:].opt()],
                    outs=[table[:].opt()],
                )

            # src-row indices for every window, resident in SBUF for the
            # (static) gather streams of all three layers
            esrc_sb = cp.tile([128, NWIN * 2 * TT], dt.int16, tag="esrc_sb")
            nc.sync.dma_start(esrc_sb[:], esrcd[:])

            def gather_phase():
                # statically unrolled indirect gathers (walrus cannot codegen
                # indirect DMA inside hardware loops): table rows by src for
                # every window, staged to DRAM for the compute loop
                for w in range(NWIN):
                    hsg = wp.tile([128, TT, 128], dt.bfloat16, tag="hsg")
                    srw = esrc_sb[:, w * 2 * TT : (w + 1) * 2 * TT].bitcast(dt.int32)
                    for t in range(TT):
                        nc.gpsimd.indirect_dma_start(
                            out=hsg[:, t, :], out_offset=None, in_=table[:],
                            in_offset=bass.IndirectOffsetOnAxis(
                                ap=srw[:, t : t + 1], axis=0
                            ),
                        )
                    nc.sync.dma_start(
                        hstg[:, w * TT * 128 : (w + 1) * TT * 128],
                        hsg[:].rearrange("p t c -> p (t c)"),
                    )

            # ---- dense phase, layer 1 ----
            with tc.For_i(0, NWIN, 1) as iv:
                xc = wp.tile([128, 128], dt.bfloat16, tag="xc")
                nc.sync.dma_start(xc[:], xT1[:, ts(iv, 128)])
                h_ps = pm_.tile([128, 64], dt.float32, tag="h_ps", space="PSUM")
                nc.tensor.matmul(out=h_ps[:], lhsT=xc[:], rhs=Ws[1][:], start=True, stop=True)
                h_sb = sp.tile([128, 64], dt.float32, tag="h_sb")
                nc.vector.tensor_copy(h_sb[:], h_ps[:])
                table_row_write(iv, 1, h_sb)
            allgather()

            # ---- edge phase per layer (layer l+1 dense fused in) ----
            for l in (1, 2, 3):
                gather_phase()
                with tc.For_i(0, NWIN, 1) as iv:
                    drel = wp.tile([128, TT], dt.bfloat16, tag="drel")
                    nc.sync.dma_start(drel[:], edrld[:, ts(iv, TT)])
                    hsrc = wp.tile([128, TT, 128], dt.bfloat16, tag="hsrc")
                    nc.sync.dma_start(
                        hsrc[:].rearrange("p t c -> p (t c)"),
                        hstg[:, ts(iv, TT * 128)],
                    )
                    S3 = wp.tile([128, TT, W], dt.bfloat16, tag="S3")
                    nc.vector.tensor_tensor(
                        out=S3[:],
                        in0=drel[:].to_broadcast([128, TT, W]),
                        in1=iota[:].unsqueeze(1).to_broadcast([128, TT, W]),
                        op=ALU.is_equal,
                    )
                    adrep = wp.tile([128, 4, 128], dt.bfloat16, tag="adrep")
                    nc.sync.dma_start(
                        adrep[:],
                        adtab[:, ts(iv, 128)].unsqueeze(0).to_broadcast([128, 4, 128]),
                    )
                    ade = wp.tile([128, TT, 4], dt.float32, tag="ade")
                    tmph = wp.tile([128, TT, 2, 128], dt.bfloat16, tag="tmph")
                    for h in range(0, 4, 2):
                        nc.vector.tensor_tensor(
                            out=tmph[:],
                            in0=S3[:].unsqueeze(2).to_broadcast([128, TT, 2, 128]),
                            in1=adrep[:, h : h + 2, :].unsqueeze(1).to_broadcast(
                                [128, TT, 2, 128]
                            ),
                            op=ALU.mult,
                        )
                        nc.vector.tensor_reduce(
                            ade[:, :, h : h + 2], tmph[:], axis=AX.X, op=ALU.add,
                        )
                    e_sb = sp.tile([128, TT * 4], dt.float32, tag="e_sb")
                    nc.vector.tensor_tensor(
                        out=e_sb[:].rearrange("p (t f) -> p t f", f=4),
                        in0=hsrc[:, :, 64:72].bitcast(dt.float32),
                        in1=ade[:],
                        op=ALU.add,
                    )
                    nc.vector.scalar_tensor_tensor(
                        out=e_sb[:], in0=e_sb[:], scalar=0.2, in1=e_sb[:],
                        op0=ALU.mult, op1=ALU.max,
                    )
                    wmsg = wp.tile([128, TT, 72], dt.bfloat16, tag="wmsg")
                    nc.scalar.activation(
                        wmsg[:, :, 64:68], e_sb[:].rearrange("p (t f) -> p t f", f=4),
                        AF.Exp,
                    )
                    nc.vector.tensor_tensor(
                        out=wmsg[:, :, 0:64].rearrange("p t (h c) -> p t h c", c=16),
                        in0=hsrc[:, :, 0:64].rearrange("p t (h c) -> p t h c", c=16),
                        in1=wmsg[:, :, 64:68].unsqueeze(3).to_broadcast([128, TT, 4, 16]),
                        op=ALU.mult,
                    )
                    out_ps = pa.tile([128, 68], dt.float32, tag="out_ps", space="PSUM")
                    for t in range(TT):
                        nc.tensor.matmul(
                            out=out_ps[:],
                            lhsT=S3[:, t, :],
                            rhs=wmsg[:, t, 0:68],
                            start=(t == 0), stop=(t == TT - 1),
                        )
                    # node phase
                    sg = sp.tile([128, 4], dt.float32, tag="sg")
                    nc.vector.tensor_scalar_add(sg[:], out_ps[:, 64:68], 1e-30)
                    rs = sp.tile([128, 4], dt.float32, tag="rs")
                    nc.vector.reciprocal(rs[:], sg[:])
                    xn = sp.tile([128, 64], dt.float32, tag="xn")
                    nc.vector.tensor_tensor(
                        out=xn[:].rearrange("p (h c) -> p h c", c=16),
                        in0=out_ps[:, 0:64].rearrange("p (h c) -> p h c", c=16),
                        in1=rs[:].unsqueeze(2).to_broadcast([128, 4, 16]),
                        op=ALU.mult,
                    )
                    nc.vector.tensor_add(xn[:], xn[:], bs[l][:])
                    nc.scalar.activation(xn[:], xn[:], AF.Tanh)
                    if l < 3:
                        # fused dense for layer l+1
                        xt_ps = pm_.tile([64, 128], dt.float32, tag="xt_ps", space="PSUM")
                        nc.tensor.transpose(out=xt_ps[:], in_=xn[:], identity=idf32[:])
                        xt_sb = sp.tile([64, 128], dt.bfloat16, tag="xt_sb")
                        nc.vector.tensor_copy(xt_sb[:], xt_ps[:])
                        h2_ps = pm_.tile([128, 64], dt.float32, tag="h_ps", space="PSUM")
                        nc.tensor.matmul(
                            out=h2_ps[:], lhsT=xt_sb[:], rhs=Ws[l + 1][:],
                            start=True, stop=True,
                        )
                        h2_sb = sp.tile([128, 64], dt.float32, tag="h_sb")
                        nc.vector.tensor_copy(h2_sb[:], h2_ps[:])
                        table_row_write(iv, l + 1, h2_sb)
                    else:
                        hm = sp.tile([128, 16], dt.float32, tag="hm")
                        nc.vector.tensor_reduce(
                            hm[:], xn[:].rearrange("p (h c) -> p c h", c=16),
                            axis=AX.X, op=ALU.add,
                        )
                        hm_ps = pm_.tile([16, 128], dt.float32, tag="xt_ps", space="PSUM")
                        nc.tensor.transpose(out=hm_ps[:], in_=hm[:], identity=idf32[:])
                        hm_sb = sp.tile([16, 128], dt.float32, tag="hm_sb")
                        nc.vector.tensor_copy(hm_sb[:], hm_ps[:])
                        nc.sync.dma_start(hmD[:, ts(iv, 128)], hm_sb[:])
                if l < 3:
                    allgather()

        # ---- readout (own pool scope; SBUF from the layer phase is freed) ----
        with (
            tc.tile_pool(name="ro", bufs=1) as cp,
            tc.tile_pool(name="ros", bufs=2) as sp,
            tc.tile_pool(name="rop", bufs=2, space="PSUM") as pm_,
        ):
            idro = cp.tile([16, 16], dt.float32, tag="idro")
            nc.sync.dma_start(idro[:], idf32d[0:16, 0:16])
            hmT = cp.tile([16, NLOC], dt.float32, tag="hmT")
            nc.sync.dma_start(hmT[:], hmD[:])
            z16 = cp.tile([16, NLOC], dt.bfloat16, tag="z16")
            r16 = cp.tile([16, NLOC], dt.bfloat16, tag="r16")
            nc.sync.dma_start(z16[:], z16d[0:1, :].to_broadcast([16, NLOC]))
            nc.sync.dma_start(r16[:], r16d[0:1, :].to_broadcast([16, NLOC]))
            gsumT = cp.tile([16, NLOC], dt.float32, tag="gsumT")
            gmaxT = cp.tile([16, NLOC], dt.float32, tag="gmaxT")
            nc.vector.tensor_tensor_scan(
                out=gsumT[:], data0=z16[:], data1=hmT[:], initial=0.0,
                op0=ALU.mult, op1=ALU.add,
            )
            nc.vector.tensor_tensor_scan(
                out=gmaxT[:], data0=r16[:], data1=hmT[:], initial=-1e30,
                op0=ALU.add, op1=ALU.max,
            )
            for w in range(NWIN):
                for (scanT, stage, tg) in ((gsumT, gsD, "s"), (gmaxT, gmD, "m")):
                    tp = pm_.tile([128, 16], dt.float32, tag="rops" + tg, space="PSUM")
                    nc.tensor.transpose(
                        out=tp[:], in_=scanT[:, w * 128 : (w + 1) * 128],
                        identity=idro[:],
                    )
                    tsb = sp.tile([128, 16], dt.float32, tag="tsb" + tg)
                    nc.vector.tensor_copy(tsb[:], tp[:])
                    nc.sync.dma_start(stage[w * 128 : (w + 1) * 128, :], tsb[:])

            wmax = cp.tile([64, 16], dt.float32, tag="wmax")
            wmean = cp.tile([64, 16], dt.float32, tag="wmean")
            wsum = cp.tile([64, 16], dt.float32, tag="wsum")
            bout = cp.tile([64, 1], dt.float32, tag="bout")
            gend = cp.tile([64, 1], dt.int32, tag="gend")
            invc = cp.tile([64, 1], dt.float32, tag="invc")
            nc.sync.dma_start(wmax[:], wmaxd[:])
            nc.sync.dma_start(wmean[:], wmeand[:])
            nc.sync.dma_start(wsum[:], wsumd[:])
            nc.sync.dma_start(bout[:], boutd[:])
            nc.sync.dma_start(gend[:], gendd[:])
            nc.sync.dma_start(invc[:], invcd[:])
            gsE = sp.tile([64, 16], dt.float32, tag="gsE")
            gmE = sp.tile([64, 16], dt.float32, tag="gmE")
            nc.gpsimd.indirect_dma_start(
                out=gsE[:], out_offset=None, in_=gsD[:],
                in_offset=bass.IndirectOffsetOnAxis(ap=gend[:], axis=0),
            )
            nc.gpsimd.indirect_dma_start(
                out=gmE[:], out_offset=None, in_=gmD[:],
                in_offset=bass.IndirectOffsetOnAxis(ap=gend[:], axis=0),
            )
            acc = sp.tile([64, 16], dt.float32, tag="acc")
            tmp2 = sp.tile([64, 16], dt.float32, tag="tmp2")
            # acc = gmax*wmax + gsum*wsum + gsum*invc*wmean  (x0.25 at the end)
            nc.vector.tensor_mul(acc[:], gmE[:], wmax[:])
            nc.vector.tensor_mul(tmp2[:], gsE[:], wsum[:])
            nc.vector.tensor_add(acc[:], acc[:], tmp2[:])
            nc.vector.tensor_mul(tmp2[:], gsE[:], wmean[:])
            nc.vector.tensor_mul(tmp2[:], tmp2[:], invc[:].to_broadcast([64, 16]))
            nc.vector.tensor_add(acc[:], acc[:], tmp2[:])
            osum = sp.tile([64, 1], dt.float32, tag="osum")
            nc.vector.tensor_reduce(osum[:], acc[:], axis=AX.X, op=ALU.add)
            o_sb = sp.tile([64, 1], dt.float32, tag="o_sb")
            nc.vector.tensor_scalar_mul(o_sb[:], osum[:], 0.25)
            nc.vector.tensor_add(o_sb[:], o_sb[:], bout[:])
            nc.sync.dma_start(outd[:], o_sb[:])

    split_waits(nc)
    return nc


def prep_params(d):
    """Replicated parameter tensors (same for all cores)."""
    out = {}
    out["iota"] = np.arange(W, dtype=np.float32).reshape(1, W).astype(
        ml_dtypes.bfloat16
    )
    out["idf32"] = np.eye(128, dtype=np.float32)
    for l, fin in ((1, 128), (2, HC), (3, HC)):
        Wl = np.asarray(d[f"W{l}"], np.float32)
        out[f"W{l}"] = Wl.astype(ml_dtypes.bfloat16)
        out[f"asrep{l}"] = np.asarray(d[f"as{l}"], np.float32).reshape(1, HC)
        out[f"adrep{l}"] = np.asarray(d[f"ad{l}"], np.float32).reshape(1, HC)
        out[f"brep{l}"] = np.asarray(d[f"b{l}"], np.float32).reshape(1, HC)
    Wout = np.asarray(d["Wout"], np.float32)
    out["wmaxr"] = np.tile(Wout[0:16].reshape(1, 16), (64, 1))
    out["wmeanr"] = np.tile(Wout[16:32].reshape(1, 16), (64, 1))
    out["wsumr"] = np.tile(Wout[32:48].reshape(1, 16), (64, 1))
    out["boutr"] = np.full((64, 1), np.float32(np.asarray(d["bout"]).reshape(-1)[0]))
    return out


def make_in_maps(d, t_tiles=T):
    inputs, _ = prep(d["x"], d["edge_index"], d["batch_index"], t_tiles)
    params = prep_params(d)
    maps = []
    for c in range(NC):
        m = dict(inputs[c])
        m.update(params)
        maps.append(m)
    return maps


_CACHE = {}


def kernel(**inputs) -> np.ndarray:
    d = {k: np.asarray(v) for k, v in inputs.items()}
    t_tiles = T
    while True:
        try:
            maps = make_in_maps(d, t_tiles)
            break
        except OverflowError as e:
            t_tiles = -(-int(e.args[0]) // 128)
    if t_tiles not in _CACHE:
        _CACHE[t_tiles] = build(NC, t_tiles)
    nc = _CACHE[t_tiles]
    res = run_bass_kernel_spmd(nc, maps, list(range(NC)))
    got = np.concatenate([res.results[c]["out"].reshape(-1) for c in range(NC)])
    return got.reshape(G, 1).astype(np.float32)
